# revision 3
# baseline (speedup 1.0000x reference)
"""Grouped-linear (EvolvedLoopLinear) Trainium2 Bass kernel, v3.

Problem: out[b, j] = sum_s x[b, g*64+s] * weight[j, g*64+s] + bias[j],
with g = j % 128, for x [4096, 8192], weight [4096, 8192], bias [4096].

Only a gathered [4096, 64] slice of the weight matrix is live, so the
kernel is pure memory streaming: read x, write out.  Strategy:

  - Data-parallel over batch across 8 cores (512 rows each).
  - x is packed per-core into a PE-ready transposed fp8-e3m4 layout
    xt[64h+s, 512k+b] = x[b, 64(2k+h)+s] (pair k = groups 2k,2k+1
    stacked on the 128 partitions).  e3m4 halves HBM read traffic vs
    fp16; measured end-to-end error ~1.1e-2 (gate 2e-2).
  - The live weight slice is pre-built on the host into block-diagonal
    pair stationaries (fp16, 1 MiB per core); the matmul mixes an fp16
    stationary with the fp8 moving operand.
  - Bias is added on the host during unscramble, so the PSUM
    evacuation is a pure wide copy (f32->f16, 2 supers = FD 1024 per
    instruction), alternating DVE/ACT.
  - Reads are split across both HWDGE rings (sync + scalar) so issue
    serialization doesn't throttle the early stream; stores fill in
    on both rings as evacuations complete, keeping ring bytes even.
  - Tile objects are kept to a minimum (4 rotating PSUM tiles, coarse
    slabs/otiles) because the TileContext exit barrier costs ~2
    all-engine barrier rounds per tile.
  - The transposed fp16 output [128, 512*32] is unscrambled to
    [4096, 4096] f32 (+bias) on the host.
"""
import os as _os
import numpy as np
import ml_dtypes
from contextlib import ExitStack

import concourse.bass as bass
import concourse.tile as tile
import concourse.tile_sem_assignment as _tsa
from concourse import bacc, mybir
from concourse.bass_utils import run_bass_kernel_spmd

# The walrus build in this container rejects instructions carrying more than
# a couple of semaphore waits ("Too many sync wait commands"); capping the
# HWDGE completion lanes keeps the kernel-tail drain under that limit.
_tsa.NUM_HWDGE_SEMS = int(_os.environ.get("K_HWSEMS", "8"))

BATCH = 4096
IN_F = 8192
OUT_F = 4096
GROUPS = 128
STEP = 64
M_PER_G = 32          # outputs per group
N_CORES = 8
B_CORE = BATCH // N_CORES      # 512
N_PAIR = GROUPS // 2           # 64 group pairs (k: groups 2k, 2k+1)
N_SUPER = N_PAIR // 2          # 32 supers (t: pairs 2t, 2t+1)


def _intlist(env, default):
    v = _os.environ.get(env)
    return [int(s) for s in v.split(",")] if v else default


# x slab sizes in pairs (64 KiB/pair fp8); small at the head (fast start)
# and tail (short last-matmul chain), big in the middle (few issues)
SLAB_PAIRS = _intlist("K_SLABS", [4, 4, 8, 12, 16, 12, 4, 2, 2])
assert sum(SLAB_PAIRS) == N_PAIR
# slab indices whose load rides the scalar (ACT) HWDGE ring
SCALAR_SLABS = set(_intlist("K_SCALAR_SLABS", [1, 3, 5]))
# w chunk sizes in pairs (both ride sync, first chunk ahead of slab0)
WCHUNK_PAIRS = _intlist("K_WCHUNKS", [8, 56])
assert sum(WCHUNK_PAIRS) == N_PAIR
# out tile sizes in supers
OTILE_SUPERS = _intlist("K_OTILES", [8, 8, 8, 4, 2, 2])
assert sum(OTILE_SUPERS) == N_SUPER
assert all(s % 2 == 0 for s in OTILE_SUPERS)
# otile indices whose store rides the sync ring (SP issues them promptly;
# balances ring bytes: sync 3.5r+1w vs scalar 1.5r+3.1w MiB)
SYNC_OTILES = set(_intlist("K_SYNC_OTILES", [3, 4, 5]))

f32 = mybir.dt.float32
f16 = mybir.dt.float16
f8e3 = mybir.dt.float8e3

WARMUP_MM = int(_os.environ.get("K_WARMUP", "8"))
WARMUP_N = int(_os.environ.get("K_WARMUP_N", "64"))
N_PS = int(_os.environ.get("K_NPS", "4"))

_COMPILED = {}


def _build():
    if "nc" in _COMPILED:
        return _COMPILED["nc"]

    nc = bacc.Bacc("TRN2", target_bir_lowering=False, debug=False)
    x_ap = nc.dram_tensor("x_s", [128, N_PAIR * B_CORE], f8e3,
                          kind="ExternalInput").ap()
    w_ap = nc.dram_tensor("w_s", [128, N_PAIR * 64], f16,
                          kind="ExternalInput").ap()
    y_ap = nc.dram_tensor("out_s", [128, N_SUPER * B_CORE], f16,
                          kind="ExternalOutput").ap()

    with tile.TileContext(nc) as tc:
        with ExitStack() as ctx:
            const_pool = ctx.enter_context(tc.tile_pool(name="const", bufs=1))
            w_pool = ctx.enter_context(
                tc.tile_pool(name="w", bufs=len(WCHUNK_PAIRS)))
            slab_pool = ctx.enter_context(
                tc.tile_pool(name="slab", bufs=len(SLAB_PAIRS)))
            osb_pool = ctx.enter_context(
                tc.tile_pool(name="osb", bufs=len(OTILE_SUPERS)))
            ps_pool = ctx.enter_context(tc.tile_pool(name="ps", bufs=N_PS,
                                                     space="PSUM"))

            w_of_pair = {}
            wp0 = 0
            w_tiles = []
            for wi, npw in enumerate(WCHUNK_PAIRS):
                wt = w_pool.tile([128, npw * 64], f16, tag="w",
                                 name=f"w{wi}")
                w_tiles.append((wt, wp0, npw))
                for j in range(npw):
                    w_of_pair[wp0 + j] = (wt, j * 64)
                wp0 += npw

            slab_of_pair = {}
            sp0 = 0
            slab_tiles = []
            for si, nps_ in enumerate(SLAB_PAIRS):
                s = slab_pool.tile([128, nps_ * B_CORE], f8e3, tag="slab",
                                   name=f"slab{si}")
                slab_tiles.append((s, sp0, nps_, si))
                for j in range(nps_):
                    slab_of_pair[sp0 + j] = (s, j * B_CORE)
                sp0 += nps_

            # issue order: sync gets w0, slab0, w1, then its slabs;
            # scalar-ring slabs are issued by ACT concurrently
            wt, wp, npw = w_tiles[0]
            nc.sync.dma_start(wt[:], w_ap[:, wp * 64:(wp + npw) * 64])
            for s, sp, nps_, si in slab_tiles:
                if si in SCALAR_SLABS:
                    nc.scalar.dma_start(
                        s[:], x_ap[:, sp * B_CORE:(sp + nps_) * B_CORE])
            for wt, wp, npw in w_tiles[1:]:
                nc.sync.dma_start(wt[:], w_ap[:, wp * 64:(wp + npw) * 64])
            for s, sp, nps_, si in slab_tiles:
                if si not in SCALAR_SLABS:
                    nc.sync.dma_start(
                        s[:], x_ap[:, sp * B_CORE:(sp + nps_) * B_CORE])

            # rotating PSUM tiles (preallocated: tile count drives the exit
            # barrier cost)
            ps_tiles = [ps_pool.tile([128, 2 * B_CORE], f32, tag="ps",
                                     name=f"ps{i}") for i in range(N_PS)]

            if WARMUP_MM:
                # pull the PE activity monitor up before the first real
                # matmuls issue
                warm = const_pool.tile([128, WARMUP_N], f16)
                nc.gpsimd.memset(warm[:], 0)
                for _ in range(WARMUP_MM):
                    nc.tensor.matmul(ps_tiles[0][0:64, 0:WARMUP_N],
                                     warm[:, 0:64], warm[:],
                                     start=True, stop=True)

            evac_i = 0
            psi = 0
            t0 = 0
            for C, ns_ in enumerate(OTILE_SUPERS):
                ot = osb_pool.tile([128, ns_ * B_CORE], f16, tag="osb",
                                   name=f"osb{C}")
                for half in range(ns_ // 2):
                    ps = ps_tiles[psi % N_PS]
                    psi += 1
                    for t2 in range(2):
                        t = t0 + 2 * half + t2
                        for u in range(2):
                            k = 2 * t + u
                            slab, soff = slab_of_pair[k]
                            wt, woff = w_of_pair[k]
                            # column-tiling: the two pair matmuls stream
                            # concurrently on disjoint PE column groups
                            nc.tensor.matmul(
                                ps[64 * u:64 * u + 64,
                                   t2 * B_CORE:(t2 + 1) * B_CORE],
                                wt[:, woff:woff + 64],
                                slab[:, soff:soff + B_CORE],
                                start=True, stop=True,
                                tile_position=(0, 64 * u))
                    # pure wide copy evacuation (bias on host), f32 PSUM
                    # -> f16 SBUF, alternating DVE / ACT
                    dst = ot[:, (2 * half) * B_CORE:(2 * half + 2) * B_CORE]
                    if evac_i % 2 == 0:
                        nc.vector.tensor_copy(dst, ps[:])
                    else:
                        nc.scalar.copy(dst, ps[:])
                    evac_i += 1
                oeng = nc.sync if C in SYNC_OTILES else nc.scalar
                oeng.dma_start(
                    y_ap[:, t0 * B_CORE:(t0 + ns_) * B_CORE], ot[:])
                t0 += ns_

    nc.compile()
    _COMPILED["nc"] = nc
    return nc


def _prep_in_maps(x, weight, bias):
    x = np.asarray(x, dtype=np.float32)
    weight = np.asarray(weight, dtype=np.float32)

    # x -> per-core PE-ready transposed fp8 e3m4: xt[c][64h+s, 512k+b]
    #    = x[512c+b, 64(2k+h)+s]
    xt = x.reshape(N_CORES, B_CORE, N_PAIR, 2, STEP)    # [c, b, k, h, s]
    xt = xt.transpose(0, 3, 4, 2, 1)                    # [c, h, s, k, b]
    xt = np.ascontiguousarray(xt).astype(ml_dtypes.float8_e3m4)
    xt = xt.reshape(N_CORES, 128, N_PAIR * B_CORE)

    # gathered weight slice: Wg[j, s] = weight[j, (j%128)*64 + s]
    j = np.arange(OUT_F)
    Wg = weight.reshape(OUT_F, GROUPS, STEP)[j, j % GROUPS]   # [4096, 64]
    Wk = Wg.reshape(M_PER_G, N_PAIR, 2, STEP)                 # [m, k, h, s]
    # block-diagonal pair stationaries:
    # wbd[64h+s, 64k+32h'+m] = (h==h') * Wk[m, k, h, s]
    wbd = np.zeros((2, STEP, N_PAIR, 2, M_PER_G), dtype=np.float16)
    for h in range(2):
        wbd[h, :, :, h, :] = Wk[:, :, h, :].transpose(2, 1, 0)  # [s, k, m]
    w_s = np.ascontiguousarray(wbd.reshape(128, N_PAIR * 64))

    in_maps = []
    for c in range(N_CORES):
        in_maps.append({
            "x_s": xt[c],
            "w_s": w_s,
        })
    return in_maps


def _unscramble(results, bias):
    # y[64u+32h+m, 512t+b] = out[512c+b, m*128 + 4t + 2u + h] - bias
    bias = np.asarray(bias, dtype=np.float32)
    out = np.empty((BATCH, OUT_F), dtype=np.float32)
    for c in range(N_CORES):
        y = np.asarray(results[c]["out_s"])                  # [128, 16384] f16
        o = y.reshape(2, 2, M_PER_G, N_SUPER, B_CORE)        # [u, h, m, t, b]
        o = o.transpose(4, 2, 3, 0, 1)                       # [b, m, t, u, h]
        out[c * B_CORE:(c + 1) * B_CORE] = o.reshape(B_CORE, OUT_F)
    out += bias
    return out


def kernel(x, weight, bias):
    nc = _build()
    in_maps = _prep_in_maps(x, weight, bias)
    res = run_bass_kernel_spmd(nc, in_maps, core_ids=list(range(N_CORES)))
    return _unscramble(res.results, bias)


# revision 5
# speedup vs baseline: 1.0575x; 1.0575x over previous
"""Grouped-linear (EvolvedLoopLinear) Trainium2 Bass kernel, v3.

Problem: out[b, j] = sum_s x[b, g*64+s] * weight[j, g*64+s] + bias[j],
with g = j % 128, for x [4096, 8192], weight [4096, 8192], bias [4096].

Only a gathered [4096, 64] slice of the weight matrix is live, so the
kernel is pure memory streaming: read x, write out.  Strategy:

  - Data-parallel over batch across 8 cores (512 rows each).
  - x is packed per-core into a PE-ready transposed fp8-e3m4 layout
    xt[64h+s, 512k+b] = x[b, 64(2k+h)+s] (pair k = groups 2k,2k+1
    stacked on the 128 partitions).  e3m4 halves HBM read traffic vs
    fp16; measured end-to-end error ~1.1e-2 (gate 2e-2).
  - The live weight slice is pre-built on the host into block-diagonal
    pair stationaries (fp16, 1 MiB per core); the matmul mixes an fp16
    stationary with the fp8 moving operand.
  - Bias is added on the host during unscramble, so the PSUM
    evacuation is a pure wide copy (f32->f16, 2 supers = FD 1024 per
    instruction), alternating DVE/ACT.
  - Reads are split across both HWDGE rings (sync + scalar) so issue
    serialization doesn't throttle the early stream; stores fill in
    on both rings as evacuations complete, keeping ring bytes even.
  - Tile objects are kept to a minimum (4 rotating PSUM tiles, coarse
    slabs/otiles) because the TileContext exit barrier costs ~2
    all-engine barrier rounds per tile.
  - The transposed fp16 output [128, 512*32] is unscrambled to
    [4096, 4096] f32 (+bias) on the host.
"""
import os as _os
import numpy as np
import ml_dtypes
from contextlib import ExitStack

import concourse.bass as bass
import concourse.tile as tile
import concourse.tile_sem_assignment as _tsa
from concourse import bacc, mybir
from concourse.bass_utils import run_bass_kernel_spmd

# The walrus build in this container rejects instructions carrying more than
# a couple of semaphore waits ("Too many sync wait commands"); capping the
# HWDGE completion lanes keeps the kernel-tail drain under that limit.
_tsa.NUM_HWDGE_SEMS = int(_os.environ.get("K_HWSEMS", "8"))

BATCH = 4096
IN_F = 8192
OUT_F = 4096
GROUPS = 128
STEP = 64
M_PER_G = 32          # outputs per group
N_CORES = 8
B_CORE = BATCH // N_CORES      # 512
N_PAIR = GROUPS // 2           # 64 group pairs (k: groups 2k, 2k+1)
N_SUPER = N_PAIR // 2          # 32 supers (t: pairs 2t, 2t+1)


def _intlist(env, default):
    v = _os.environ.get(env)
    return [int(s) for s in v.split(",")] if v else default


# x slab sizes in pairs (64 KiB/pair fp8); small at the head (fast start)
# and tail (short last-matmul chain), big in the middle (few issues)
SLAB_PAIRS = _intlist("K_SLABS", [4, 4, 8, 12, 16, 12, 4, 2, 2])
assert sum(SLAB_PAIRS) == N_PAIR
# slab indices whose load rides the scalar (ACT) HWDGE ring
SCALAR_SLABS = set(_intlist("K_SCALAR_SLABS", [1, 3, 5]))
# w chunk sizes in pairs (all ride sync, interleaved with the sync slabs
# so each chunk lands just before the slabs that need it)
WCHUNK_PAIRS = _intlist("K_WCHUNKS", [8, 8, 16, 32])
assert sum(WCHUNK_PAIRS) == N_PAIR
# out tile sizes in supers
OTILE_SUPERS = _intlist("K_OTILES", [8, 8, 8, 4, 2, 2])
assert sum(OTILE_SUPERS) == N_SUPER
assert all(s % 2 == 0 for s in OTILE_SUPERS)
# otile indices whose store rides the sync ring (SP issues them promptly;
# balances ring bytes: sync 3.5r+1w vs scalar 1.5r+3.1w MiB)
SYNC_OTILES = set(_intlist("K_SYNC_OTILES", [3, 4, 5]))

f32 = mybir.dt.float32
f16 = mybir.dt.float16
f8e3 = mybir.dt.float8e3

WARMUP_MM = int(_os.environ.get("K_WARMUP", "8"))
WARMUP_N = int(_os.environ.get("K_WARMUP_N", "64"))
N_PS = int(_os.environ.get("K_NPS", "4"))

_COMPILED = {}


def _build():
    if "nc" in _COMPILED:
        return _COMPILED["nc"]

    nc = bacc.Bacc("TRN2", target_bir_lowering=False, debug=False)
    x_ap = nc.dram_tensor("x_s", [128, N_PAIR * B_CORE], f8e3,
                          kind="ExternalInput").ap()
    w_ap = nc.dram_tensor("w_s", [128, N_PAIR * 64], f16,
                          kind="ExternalInput").ap()
    y_ap = nc.dram_tensor("out_s", [128, N_SUPER * B_CORE], f16,
                          kind="ExternalOutput").ap()

    with tile.TileContext(nc) as tc:
        with ExitStack() as ctx:
            const_pool = ctx.enter_context(tc.tile_pool(name="const", bufs=1))
            w_pool = ctx.enter_context(
                tc.tile_pool(name="w", bufs=len(WCHUNK_PAIRS)))
            slab_pool = ctx.enter_context(
                tc.tile_pool(name="slab", bufs=len(SLAB_PAIRS)))
            osb_pool = ctx.enter_context(
                tc.tile_pool(name="osb", bufs=len(OTILE_SUPERS)))
            ps_pool = ctx.enter_context(tc.tile_pool(name="ps", bufs=N_PS,
                                                     space="PSUM"))

            w_of_pair = {}
            wp0 = 0
            w_tiles = []
            for wi, npw in enumerate(WCHUNK_PAIRS):
                wt = w_pool.tile([128, npw * 64], f16, tag="w",
                                 name=f"w{wi}")
                w_tiles.append((wt, wp0, npw))
                for j in range(npw):
                    w_of_pair[wp0 + j] = (wt, j * 64)
                wp0 += npw

            slab_of_pair = {}
            sp0 = 0
            slab_tiles = []
            for si, nps_ in enumerate(SLAB_PAIRS):
                s = slab_pool.tile([128, nps_ * B_CORE], f8e3, tag="slab",
                                   name=f"slab{si}")
                slab_tiles.append((s, sp0, nps_, si))
                for j in range(nps_):
                    slab_of_pair[sp0 + j] = (s, j * B_CORE)
                sp0 += nps_

            # scalar-ring slab loads are issued by ACT, concurrently with
            # the sync stream below
            for s, sp, nps_, si in slab_tiles:
                if si in SCALAR_SLABS:
                    nc.scalar.dma_start(
                        s[:], x_ap[:, sp * B_CORE:(sp + nps_) * B_CORE])
            # sync-ring order: each w chunk goes out just before the first
            # sync slab whose pairs need it, so neither blocks the other
            # for long (the sync ring is a FIFO)
            wq = list(w_tiles)
            for s, sp, nps_, si in slab_tiles:
                if si in SCALAR_SLABS:
                    continue
                while wq and wq[0][1] < sp + nps_:
                    wt, wp, npw = wq.pop(0)
                    nc.sync.dma_start(
                        wt[:], w_ap[:, wp * 64:(wp + npw) * 64])
                nc.sync.dma_start(
                    s[:], x_ap[:, sp * B_CORE:(sp + nps_) * B_CORE])
            for wt, wp, npw in wq:
                nc.sync.dma_start(wt[:], w_ap[:, wp * 64:(wp + npw) * 64])

            # rotating PSUM tiles (preallocated: tile count drives the exit
            # barrier cost)
            ps_tiles = [ps_pool.tile([128, 2 * B_CORE], f32, tag="ps",
                                     name=f"ps{i}") for i in range(N_PS)]

            if WARMUP_MM:
                # pull the PE activity monitor up before the first real
                # matmuls issue
                warm = const_pool.tile([128, WARMUP_N], f16)
                nc.gpsimd.memset(warm[:], 0)
                for _ in range(WARMUP_MM):
                    nc.tensor.matmul(ps_tiles[0][0:64, 0:WARMUP_N],
                                     warm[:, 0:64], warm[:],
                                     start=True, stop=True)

            evac_i = 0
            psi = 0
            t0 = 0
            for C, ns_ in enumerate(OTILE_SUPERS):
                ot = osb_pool.tile([128, ns_ * B_CORE], f16, tag="osb",
                                   name=f"osb{C}")
                for half in range(ns_ // 2):
                    ps = ps_tiles[psi % N_PS]
                    psi += 1
                    for t2 in range(2):
                        t = t0 + 2 * half + t2
                        for u in range(2):
                            k = 2 * t + u
                            slab, soff = slab_of_pair[k]
                            wt, woff = w_of_pair[k]
                            # column-tiling: the two pair matmuls stream
                            # concurrently on disjoint PE column groups
                            nc.tensor.matmul(
                                ps[64 * u:64 * u + 64,
                                   t2 * B_CORE:(t2 + 1) * B_CORE],
                                wt[:, woff:woff + 64],
                                slab[:, soff:soff + B_CORE],
                                start=True, stop=True,
                                tile_position=(0, 64 * u))
                    # pure wide copy evacuation (bias on host), f32 PSUM
                    # -> f16 SBUF, alternating DVE / ACT
                    dst = ot[:, (2 * half) * B_CORE:(2 * half + 2) * B_CORE]
                    if evac_i % 2 == 0:
                        nc.vector.tensor_copy(dst, ps[:])
                    else:
                        nc.scalar.copy(dst, ps[:])
                    evac_i += 1
                oeng = nc.sync if C in SYNC_OTILES else nc.scalar
                oeng.dma_start(
                    y_ap[:, t0 * B_CORE:(t0 + ns_) * B_CORE], ot[:])
                t0 += ns_

    nc.compile()
    _COMPILED["nc"] = nc
    return nc


def _prep_in_maps(x, weight, bias):
    x = np.asarray(x, dtype=np.float32)
    weight = np.asarray(weight, dtype=np.float32)

    # x -> per-core PE-ready transposed fp8 e3m4: xt[c][64h+s, 512k+b]
    #    = x[512c+b, 64(2k+h)+s]
    xt = x.reshape(N_CORES, B_CORE, N_PAIR, 2, STEP)    # [c, b, k, h, s]
    xt = xt.transpose(0, 3, 4, 2, 1)                    # [c, h, s, k, b]
    xt = np.ascontiguousarray(xt).astype(ml_dtypes.float8_e3m4)
    xt = xt.reshape(N_CORES, 128, N_PAIR * B_CORE)

    # gathered weight slice: Wg[j, s] = weight[j, (j%128)*64 + s]
    j = np.arange(OUT_F)
    Wg = weight.reshape(OUT_F, GROUPS, STEP)[j, j % GROUPS]   # [4096, 64]
    Wk = Wg.reshape(M_PER_G, N_PAIR, 2, STEP)                 # [m, k, h, s]
    # block-diagonal pair stationaries:
    # wbd[64h+s, 64k+32h'+m] = (h==h') * Wk[m, k, h, s]
    wbd = np.zeros((2, STEP, N_PAIR, 2, M_PER_G), dtype=np.float16)
    for h in range(2):
        wbd[h, :, :, h, :] = Wk[:, :, h, :].transpose(2, 1, 0)  # [s, k, m]
    w_s = np.ascontiguousarray(wbd.reshape(128, N_PAIR * 64))

    in_maps = []
    for c in range(N_CORES):
        in_maps.append({
            "x_s": xt[c],
            "w_s": w_s,
        })
    return in_maps


def _unscramble(results, bias):
    # y[64u+32h+m, 512t+b] = out[512c+b, m*128 + 4t + 2u + h] - bias
    bias = np.asarray(bias, dtype=np.float32)
    out = np.empty((BATCH, OUT_F), dtype=np.float32)
    for c in range(N_CORES):
        y = np.asarray(results[c]["out_s"])                  # [128, 16384] f16
        o = y.reshape(2, 2, M_PER_G, N_SUPER, B_CORE)        # [u, h, m, t, b]
        o = o.transpose(4, 2, 3, 0, 1)                       # [b, m, t, u, h]
        out[c * B_CORE:(c + 1) * B_CORE] = o.reshape(B_CORE, OUT_F)
    out += bias
    return out


def kernel(x, weight, bias):
    nc = _build()
    in_maps = _prep_in_maps(x, weight, bias)
    res = run_bass_kernel_spmd(nc, in_maps, core_ids=list(range(N_CORES)))
    return _unscramble(res.results, bias)
